# revision 9
# baseline (speedup 1.0000x reference)
"""HVTSurv forward pass on 8 Trainium2 NeuronCores.

Sharding: data-parallel over windows. Phase 1 (per core: contiguous shard of
6272 tokens = 128 windows): fc1+relu, LN1-folded window attention with
relative-position bias, proj, residual, gelu -> f1 shard. Host reshuffles f1
into shuffle-window order (pure data movement). Phase 2 (per core: its 128
shuffle windows): norm1-folded shuffle attention, proj+residual, norm2-folded
MLP, residual, LN3, attention-pooling partial sums. Host combines the 8
partials and applies the tiny 512x4 head.

On-chip: activations feature-major [C, T]; matmuls in float32r (1 cyc/row);
LayerNorm stats via ones-matmul broadcast; attention computed transposed
(attnT[j,i]) so softmax denominators come from ones-matmuls and the
normalization lands on the feature-major attention output.
"""

import os
import numpy as np

NCORES = 8
N = 50176
CIN = 1024
D = 512
NH = 8
WS = 49
SHIFT = 7
SCALE = (D // NH) ** -0.5
T = N // NCORES          # 6272 tokens per core
NWIN = T // WS           # 128 windows per core
CW = 8                   # windows per chunk
CT = CW * WS             # 392 tokens per chunk
NCH = NWIN // CW         # 16 chunks
HP = NH // 2             # 4 head pairs
AW = HP * WS             # 196 free width of one window's attnT block

_cache = {}
USE_F32R = bool(int(os.environ.get("KERNEL_F32R", "0")))


def _piecewise_index(rp, alpha=1.9, beta=1.9 * 4, gamma=1.9 * 6, shift=SHIFT):
    rpf = rp.astype(np.float32)
    rp_abs = np.abs(rpf)
    mask = rp_abs <= 2.0 * alpha
    safe = np.maximum(rp_abs, 1e-6)
    y = np.sign(rpf) * np.minimum(
        np.round(np.log(safe / alpha) / np.log(gamma / alpha) * (beta - 2 * alpha)),
        float(shift),
    )
    idx = np.where(mask, np.sign(rpf), y)
    return idx.astype(np.int32)


def _host_prep(inputs):
    f32 = np.float32
    data = np.asarray(inputs["data"], dtype=f32)
    coords = data[0, :, :2]
    feats = data[0, :, 2:]

    featsT = [np.ascontiguousarray(feats[m * T:(m + 1) * T].T) for m in range(NCORES)]

    table = np.asarray(inputs["wa_bias_table"], dtype=f32)
    cw = coords.reshape(N // WS, WS, 2)
    rel = (cw[:, :, None, :] - cw[:, None, :, :]).astype(np.int32)
    idx = np.abs(_piecewise_index(rel[..., 0])) + np.abs(_piecewise_index(rel[..., 1]))
    biasv = table[idx]  # [1024, 49i, 49j, 8]
    hbias = []
    for m in range(NCORES):
        bv = biasv[m * NWIN:(m + 1) * NWIN]
        hb = np.full((NWIN, 128, AW), -100.0, dtype=f32)
        for hp in range(HP):
            hb[:, 0:WS, hp * WS:(hp + 1) * WS] = bv[:, :, :, 2 * hp].transpose(0, 2, 1)
            hb[:, 64:64 + WS, hp * WS:(hp + 1) * WS] = \
                bv[:, :, :, 2 * hp + 1].transpose(0, 2, 1)
        hbias.append(np.ascontiguousarray(hb))

    def fold(g, b, W, bb, scale=1.0):
        g = np.asarray(g, f32); b = np.asarray(b, f32)
        W = np.asarray(W, f32); bb = np.asarray(bb, f32)
        return (np.ascontiguousarray((g[:, None] * W) * scale),
                ((b @ W + bb) * scale).astype(f32))

    qkv1 = np.asarray(inputs["wa_qkv_w"], f32)
    qkv1b = np.asarray(inputs["wa_qkv_b"], f32)
    Wq1, bq1 = fold(inputs["ln1_g"], inputs["ln1_b"], qkv1[:, :D], qkv1b[:D], SCALE)
    Wk1, bk1 = fold(inputs["ln1_g"], inputs["ln1_b"], qkv1[:, D:2 * D], qkv1b[D:2 * D])
    Wv1, bv1 = fold(inputs["ln1_g"], inputs["ln1_b"], qkv1[:, 2 * D:], qkv1b[2 * D:])
    Wp1 = np.ascontiguousarray(np.asarray(inputs["wa_proj_w"], f32))
    bp1 = (np.asarray(inputs["wa_proj_b"], f32) + bv1 @ Wp1).astype(f32)

    qkv2 = np.asarray(inputs["sa_qkv_w"], f32)
    qkv2b = np.asarray(inputs["sa_qkv_b"], f32)
    Wq2, bq2 = fold(inputs["norm1_g"], inputs["norm1_b"], qkv2[:, :D], qkv2b[:D], SCALE)
    Wk2, bk2 = fold(inputs["norm1_g"], inputs["norm1_b"], qkv2[:, D:2 * D], qkv2b[D:2 * D])
    Wv2, bv2 = fold(inputs["norm1_g"], inputs["norm1_b"], qkv2[:, 2 * D:], qkv2b[2 * D:])
    Wp2 = np.ascontiguousarray(np.asarray(inputs["sa_proj_w"], f32))
    bp2 = (np.asarray(inputs["sa_proj_b"], f32) + bv2 @ Wp2).astype(f32)

    Wm1, bm1 = fold(inputs["norm2_g"], inputs["norm2_b"], inputs["mlp_w1"], inputs["mlp_b1"])
    Wm2 = np.ascontiguousarray(np.asarray(inputs["mlp_w2"], f32))
    bm2 = np.asarray(inputs["mlp_b2"], f32)

    g3 = np.asarray(inputs["norm3_g"], f32)
    b3 = np.asarray(inputs["norm3_b"], f32)
    Wa1, ba1 = fold(g3, b3, inputs["ap_w1"], inputs["ap_b1"])
    Wa2bc = np.ascontiguousarray(
        np.tile(np.asarray(inputs["ap_w2"], f32), (1, 128)))
    ba2 = float(np.asarray(inputs["ap_b2"], f32).reshape(-1)[0])

    def pbias(b):
        return np.ascontiguousarray(b.reshape(-1, 128).T.astype(f32))

    return dict(
        featsT=featsT, hbias=hbias,
        W1=np.ascontiguousarray(np.asarray(inputs["fc1_w"], f32)),
        b1=pbias(np.asarray(inputs["fc1_b"], f32)),
        Wq1=Wq1, bq1=pbias(bq1), Wk1=Wk1, bk1=pbias(bk1), Wv1=Wv1,
        Wp1=Wp1, bp1=pbias(bp1),
        Wq2=Wq2, bq2=pbias(bq2), Wk2=Wk2, bk2=pbias(bk2), Wv2=Wv2,
        Wp2=Wp2, bp2=pbias(bp2),
        Wm1=Wm1, bm1=pbias(bm1), Wm2=Wm2, bm2=pbias(bm2),
        Wa1=Wa1, ba1=pbias(ba1), Wa2bc=Wa2bc, ba2=ba2,
        g3=g3, b3=b3,
        fc2_w=np.asarray(inputs["fc2_w"], f32),
        fc2_b=np.asarray(inputs["fc2_b"], f32),
        ones=np.ones((128, 128), dtype=f32),
    )


# --------------------------------------------------------------------------
# kernel builders
# --------------------------------------------------------------------------

def _ln_stats_apply(nc, pools, mybir, x, y, ones_t, eps_t):
    """y = (x - mu) / sqrt(var + eps), feature-major x [128, 4, CT]."""
    f32 = mybir.dt.float32
    f32r = mybir.dt.float32r if USE_F32R else mybir.dt.float32
    AF = mybir.ActivationFunctionType
    OP = mybir.AluOpType
    spool, psumS = pools["spool"], pools["psumS"]

    ps_s = psumS.tile([128, CT], f32, tag="ln_ps_s")
    for kt in range(4):
        nc.tensor.matmul(ps_s[:], ones_t[:].bitcast(f32r), x[:, kt, :].bitcast(f32r),
                         start=(kt == 0), stop=(kt == 3))
    ps_q = psumS.tile([128, CT], f32, tag="ln_ps_q")
    for kt in range(4):
        sq = spool.tile([128, CT], f32, tag="ln_sq")
        nc.scalar.activation(sq[:], x[:, kt, :], AF.Square)
        nc.tensor.matmul(ps_q[:], ones_t[:].bitcast(f32r), sq[:].bitcast(f32r),
                         start=(kt == 0), stop=(kt == 3))
    mu = spool.tile([128, CT], f32, tag="ln_mu")
    nc.scalar.activation(mu[:], ps_s[:], AF.Copy, scale=1.0 / D)
    m2 = spool.tile([128, CT], f32, tag="ln_m2")
    nc.scalar.activation(m2[:], ps_q[:], AF.Copy, scale=1.0 / D)
    var = spool.tile([128, CT], f32, tag="ln_var")
    nc.vector.tensor_tensor(var[:], mu[:], mu[:], OP.mult)
    nc.vector.tensor_tensor(var[:], m2[:], var[:], OP.subtract)
    r1 = spool.tile([128, CT], f32, tag="ln_r1")
    nc.scalar.activation(r1[:], var[:], AF.Ln, bias=eps_t[:])
    nc.scalar.activation(r1[:], r1[:], AF.Exp, scale=-0.5)
    r2 = spool.tile([128, CT], f32, tag="ln_r2")
    nc.vector.tensor_tensor(r2[:], mu[:], r1[:], OP.mult)
    for kt in range(4):
        nc.vector.tensor_tensor(y[:, kt, :], x[:, kt, :], r1[:], OP.mult)
        nc.vector.tensor_tensor(y[:, kt, :], y[:, kt, :], r2[:], OP.subtract)


def _dense(nc, pools, mybir, x, W_t, nk, nm, out_cb):
    """out[mt] = sum_kt W[:,kt,mt-tile].T @ x[:,kt,:] ; evict via out_cb."""
    f32 = mybir.dt.float32
    f32r = mybir.dt.float32r if USE_F32R else mybir.dt.float32
    psumM = pools["psumM"]
    for mt in range(nm):
        ps = psumM.tile([128, CT], f32, tag="dense_ps")
        for kt in range(nk):
            nc.tensor.matmul(
                ps[:], W_t[:, kt, mt * 128:(mt + 1) * 128].bitcast(f32r),
                x[:, kt, :].bitcast(f32r), start=(kt == 0), stop=(kt == nk - 1))
        out_cb(mt, ps)


def _attn_block(nc, pools, mybir, y, names, hb_tile, f_out_cb):
    """q/k/v + windowed attnT + PV + proj for one chunk.

    y: LN'd input [128,4,CT]. hb_tile: [128, CW, AW] rel-bias (or None).
    f_out_cb(mt, ps): consume proj psum."""
    f32 = mybir.dt.float32
    f32r = mybir.dt.float32r if USE_F32R else mybir.dt.float32
    AF = mybir.ActivationFunctionType
    OP = mybir.AluOpType
    a1, spool = pools["a1"], pools["spool"]
    psumA, psumD, psumV = pools["psumA"], pools["psumD"], pools["psumV"]
    ones_t = names["ones"]

    q4 = a1.tile([128, 4, CT], f32, tag="q4")
    k4 = a1.tile([128, 4, CT], f32, tag="k4")
    for dst, Wt, bt in ((q4, names["Wq"], names["bq"]), (k4, names["Wk"], names["bk"])):
        def cb(mt, ps, dst=dst, bt=bt):
            nc.scalar.activation(dst[:, mt, :], ps[:], AF.Identity,
                                 bias=bt[:, mt:mt + 1])
        _dense(nc, pools, mybir, y, Wt, 4, 4, cb)

    # v token-major, duplicated at partition rows 0-48 and 64-112
    vt_all = a1.tile([128, CW, 512], f32, tag="vt_all")
    for w in range(CW):
        ps_v = psumV.tile([128, 512], f32, tag="ps_v")
        for kt in range(4):
            nc.tensor.matmul(ps_v[0:WS, :],
                             y[:, kt, w * WS:(w + 1) * WS].bitcast(f32r),
                             names["Wv"][:, kt, :].bitcast(f32r),
                             start=(kt == 0), stop=(kt == 3))
        nc.vector.tensor_copy(vt_all[0:WS, w, :], ps_v[0:WS, :])
        nc.sync.dma_start(vt_all[64:64 + WS, w, :], vt_all[0:WS, w, :])

    # qk -> attnT[j,i]; evict (+bias) to contiguous attn_s; batched exp
    attn_s = a1.tile([128, CW * AW], f32, tag="attn_s")
    for w in range(CW):
        a_ps = psumA.tile([128, AW], f32, tag="a_ps")
        ws = w * WS
        for hp in range(HP):
            nc.tensor.matmul(a_ps[0:WS, hp * WS:(hp + 1) * WS],
                             k4[0:64, hp, ws:ws + WS].bitcast(f32r),
                             q4[0:64, hp, ws:ws + WS].bitcast(f32r),
                             start=True, stop=True, tile_position=(0, 0))
            nc.tensor.matmul(a_ps[64:64 + WS, hp * WS:(hp + 1) * WS],
                             k4[64:128, hp, ws:ws + WS].bitcast(f32r),
                             q4[64:128, hp, ws:ws + WS].bitcast(f32r),
                             start=True, stop=True, tile_position=(64, 64))
        sl = attn_s[0:113, w * AW:(w + 1) * AW]
        if hb_tile is not None:
            nc.vector.tensor_tensor(sl, a_ps[0:113, :], hb_tile[0:113, w, :], OP.add)
        else:
            nc.vector.tensor_copy(sl, a_ps[0:113, :])

    p_s = a1.tile([128, CW * AW], f32, tag="p_s")
    nc.scalar.activation(p_s[0:113, :], attn_s[0:113, :], AF.Exp)

    ao_n = names["ao_n"]
    for w in range(CW):
        wc = w * AW
        d_ps = psumD.tile([128, AW], f32, tag="d_ps")
        nc.tensor.matmul(d_ps[0:64, :], ones_t[0:WS, 0:64].bitcast(f32r),
                         p_s[0:WS, wc:wc + AW].bitcast(f32r),
                         start=True, stop=True, tile_position=(0, 0))
        nc.tensor.matmul(d_ps[64:128, :], ones_t[64:64 + WS, 0:64].bitcast(f32r),
                         p_s[64:64 + WS, wc:wc + AW].bitcast(f32r),
                         start=True, stop=True, tile_position=(64, 64))
        rden = spool.tile([128, AW], f32, tag="rden")
        nc.vector.reciprocal_approx_fast(rden[:], d_ps[:])

        ao_ps = psumA.tile([128, AW], f32, tag="ao_ps")
        for hp in range(HP):
            nc.tensor.matmul(ao_ps[0:64, hp * WS:(hp + 1) * WS],
                             vt_all[0:WS, w, hp * 128:hp * 128 + 64].bitcast(f32r),
                             p_s[0:WS, wc + hp * WS:wc + (hp + 1) * WS].bitcast(f32r),
                             start=True, stop=True, tile_position=(0, 0))
            nc.tensor.matmul(ao_ps[64:128, hp * WS:(hp + 1) * WS],
                             vt_all[64:64 + WS, w, hp * 128 + 64:hp * 128 + 128].bitcast(f32r),
                             p_s[64:64 + WS, wc + hp * WS:wc + (hp + 1) * WS].bitcast(f32r),
                             start=True, stop=True, tile_position=(64, 64))
        nc.vector.tensor_tensor(
            ao_n[:, :, w * WS:(w + 1) * WS],
            ao_ps[:].rearrange("p (hp i) -> p hp i", hp=HP),
            rden[:].rearrange("p (hp i) -> p hp i", hp=HP),
            OP.mult)

    _dense(nc, pools, mybir, ao_n, names["Wp"], 4, 4, f_out_cb)


def _mk_pools(tc, stack):
    p = {}
    p["wpool"] = stack.enter_context(tc.tile_pool(name="wpool", bufs=1))
    p["a2"] = stack.enter_context(tc.tile_pool(name="a2", bufs=2))
    p["a1"] = stack.enter_context(tc.tile_pool(name="a1", bufs=1))
    p["spool"] = stack.enter_context(tc.tile_pool(name="spool", bufs=2))
    p["psumM"] = stack.enter_context(tc.tile_pool(name="psumM", bufs=2, space="PSUM"))
    p["psumS"] = stack.enter_context(tc.tile_pool(name="psumS", bufs=1, space="PSUM"))
    p["psumA"] = stack.enter_context(tc.tile_pool(name="psumA", bufs=1, space="PSUM"))
    p["psumD"] = stack.enter_context(tc.tile_pool(name="psumD", bufs=1, space="PSUM"))
    p["psumV"] = stack.enter_context(tc.tile_pool(name="psumV", bufs=1, space="PSUM"))
    return p


def _loaders(nc, pools, mybir):
    f32 = mybir.dt.float32
    wpool = pools["wpool"]

    def wload(dram, ko, m):
        t = wpool.tile([128, ko, m], f32, tag=dram.name)
        nc.sync.dma_start(t[:], dram.ap().rearrange("(ko p) m -> p ko m", p=128))
        return t

    def bload(dram, cols):
        t = wpool.tile([128, cols], f32, tag=dram.name)
        nc.sync.dma_start(t[:], dram[:, :])
        return t

    return wload, bload


def _build_phase1(nch):
    from concourse import bacc
    import concourse.tile as tile
    import concourse.mybir as mybir
    from contextlib import ExitStack
    f32 = mybir.dt.float32
    AF = mybir.ActivationFunctionType
    OP = mybir.AluOpType

    nc = bacc.Bacc("TRN2", target_bir_lowering=False, debug=False,
                   num_devices=NCORES)
    featsT = nc.dram_tensor("featsT", [CIN, T], f32, kind="ExternalInput")
    hbias = nc.dram_tensor("hbias", [NWIN, 128, AW], f32, kind="ExternalInput")
    dr = {}
    for nm, shp in (("W1", [CIN, D]), ("b1", [128, 4]),
                    ("Wq1", [D, D]), ("bq1", [128, 4]),
                    ("Wk1", [D, D]), ("bk1", [128, 4]),
                    ("Wv1", [D, D]),
                    ("Wp1", [D, D]), ("bp1", [128, 4]),
                    ("ones", [128, 128])):
        dr[nm] = nc.dram_tensor(nm, shp, f32, kind="ExternalInput")
    f1out = nc.dram_tensor("f1", [D, T], f32, kind="ExternalOutput")

    with tile.TileContext(nc) as tc, ExitStack() as stack:
        pools = _mk_pools(tc, stack)
        wload, bload = _loaders(nc, pools, mybir)
        W1_t = wload(dr["W1"], 8, D)
        Wq_t = wload(dr["Wq1"], 4, D)
        Wk_t = wload(dr["Wk1"], 4, D)
        Wv_t = wload(dr["Wv1"], 4, D)
        Wp_t = wload(dr["Wp1"], 4, D)
        b1_t = bload(dr["b1"], 4)
        bq_t = bload(dr["bq1"], 4)
        bk_t = bload(dr["bk1"], 4)
        bp_t = bload(dr["bp1"], 4)
        ones_t = bload(dr["ones"], 128)
        eps_t = pools["wpool"].tile([128, 1], f32, tag="eps")
        nc.vector.memset(eps_t[:], 1e-5)

        fT3 = featsT.ap().rearrange("(ko p) t -> p ko t", p=128)
        f1o = f1out.ap().rearrange("(ko p) t -> p ko t", p=128)

        for c in range(nch):
            cs = c * CT
            x8 = pools["a2"].tile([128, 8, CT], f32, tag="x8")
            nc.sync.dma_start(x8[:], fT3[:, :, cs:cs + CT])
            hb_tile = pools["a1"].tile([128, CW, AW], f32, tag="hb")
            nc.sync.dma_start(
                hb_tile[:],
                hbias.ap()[c * CW:(c + 1) * CW, :, :].rearrange("w p a -> p w a"))

            h4 = pools["a2"].tile([128, 4, CT], f32, tag="h4")

            def relu_cb(mt, ps):
                nc.scalar.activation(h4[:, mt, :], ps[:], AF.Relu,
                                     bias=b1_t[:, mt:mt + 1])
            _dense(nc, pools, mybir, x8, W1_t, 8, 4, relu_cb)

            y4 = pools["a1"].tile([128, 4, CT], f32, tag="y4")
            _ln_stats_apply(nc, pools, mybir, h4, y4, ones_t, eps_t)

            ao_n = pools["a1"].tile([128, 4, CT], f32, tag="ao_n")
            f1c = pools["a1"].tile([128, 4, CT], f32, tag="f1c")

            def f_cb(mt, ps):
                hp_t = pools["spool"].tile([128, CT], f32, tag="hp_t")
                nc.vector.tensor_tensor(hp_t[:], ps[:], h4[:, mt, :], OP.add)
                nc.scalar.activation(f1c[:, mt, :], hp_t[:], AF.Gelu,
                                     bias=bp_t[:, mt:mt + 1])

            names = dict(Wq=Wq_t, bq=bq_t, Wk=Wk_t, bk=bk_t, Wv=Wv_t,
                         Wp=Wp_t, ones=ones_t, ao_n=ao_n)
            _attn_block(nc, pools, mybir, y4, names, hb_tile, f_cb)

            nc.sync.dma_start(f1o[:, :, cs:cs + CT], f1c[:])
    nc.compile()
    return nc


def _build_phase2(nch, ba2):
    from concourse import bacc
    import concourse.tile as tile
    import concourse.mybir as mybir
    from contextlib import ExitStack
    f32 = mybir.dt.float32
    f32r = mybir.dt.float32r if USE_F32R else mybir.dt.float32
    AF = mybir.ActivationFunctionType
    OP = mybir.AluOpType

    nc = bacc.Bacc("TRN2", target_bir_lowering=False, debug=False,
                   num_devices=NCORES)
    g_in = nc.dram_tensor("g", [D, T], f32, kind="ExternalInput")
    dr = {}
    for nm, shp in (("Wq2", [D, D]), ("bq2", [128, 4]),
                    ("Wk2", [D, D]), ("bk2", [128, 4]),
                    ("Wv2", [D, D]),
                    ("Wp2", [D, D]), ("bp2", [128, 4]),
                    ("Wm1", [D, D]), ("bm1", [128, 4]),
                    ("Wm2", [D, D]), ("bm2", [128, 4]),
                    ("Wa1", [D, 256]), ("ba1", [128, 2]),
                    ("Wa2bc", [256, 128]),
                    ("ones", [128, 128])):
        dr[nm] = nc.dram_tensor(nm, shp, f32, kind="ExternalInput")
    part = nc.dram_tensor("partial", [128, 8], f32, kind="ExternalOutput")

    with tile.TileContext(nc) as tc, ExitStack() as stack:
        pools = _mk_pools(tc, stack)
        wload, bload = _loaders(nc, pools, mybir)
        Wq_t = wload(dr["Wq2"], 4, D)
        Wk_t = wload(dr["Wk2"], 4, D)
        Wv_t = wload(dr["Wv2"], 4, D)
        Wp_t = wload(dr["Wp2"], 4, D)
        Wm1_t = wload(dr["Wm1"], 4, D)
        Wm2_t = wload(dr["Wm2"], 4, D)
        Wa1_t = wload(dr["Wa1"], 4, 256)
        Wa2_t = wload(dr["Wa2bc"], 2, 128)
        bq_t = bload(dr["bq2"], 4)
        bk_t = bload(dr["bk2"], 4)
        bp_t = bload(dr["bp2"], 4)
        bm1_t = bload(dr["bm1"], 4)
        bm2_t = bload(dr["bm2"], 4)
        ba1_t = bload(dr["ba1"], 2)
        ones_t = bload(dr["ones"], 128)
        eps_t = pools["wpool"].tile([128, 1], f32, tag="eps")
        nc.vector.memset(eps_t[:], 1e-5)
        ba2_t = pools["wpool"].tile([128, 1], f32, tag="ba2")
        nc.vector.memset(ba2_t[:], float(ba2))
        pp = pools["wpool"].tile([128, 8], f32, tag="pp")
        nc.vector.memset(pp[:], 0.0)

        gT3 = g_in.ap().rearrange("(ko p) t -> p ko t", p=128)

        for c in range(nch):
            cs = c * CT
            g4 = pools["a2"].tile([128, 4, CT], f32, tag="g4")
            nc.sync.dma_start(g4[:], gT3[:, :, cs:cs + CT])

            y4 = pools["a1"].tile([128, 4, CT], f32, tag="yln")
            _ln_stats_apply(nc, pools, mybir, g4, y4, ones_t, eps_t)

            ao_n = pools["a1"].tile([128, 4, CT], f32, tag="ao_n")
            f2 = pools["a2"].tile([128, 4, CT], f32, tag="f2")

            def f2_cb(mt, ps):
                t = pools["spool"].tile([128, CT], f32, tag="ev_t")
                nc.vector.tensor_scalar_add(t[:], ps[:], bp_t[:, mt:mt + 1])
                nc.vector.tensor_tensor(f2[:, mt, :], t[:], g4[:, mt, :], OP.add)

            names = dict(Wq=Wq_t, bq=bq_t, Wk=Wk_t, bk=bk_t, Wv=Wv_t,
                         Wp=Wp_t, ones=ones_t, ao_n=ao_n)
            _attn_block(nc, pools, mybir, y4, names, None, f2_cb)

            y5 = pools["a1"].tile([128, 4, CT], f32, tag="yln")
            _ln_stats_apply(nc, pools, mybir, f2, y5, ones_t, eps_t)
            u4 = pools["a1"].tile([128, 4, CT], f32, tag="u4")

            def mlp1_cb(mt, ps):
                nc.scalar.activation(u4[:, mt, :], ps[:], AF.Gelu,
                                     bias=bm1_t[:, mt:mt + 1])
            _dense(nc, pools, mybir, y5, Wm1_t, 4, 4, mlp1_cb)

            f3 = pools["a2"].tile([128, 4, CT], f32, tag="f3")

            def mlp2_cb(mt, ps):
                t = pools["spool"].tile([128, CT], f32, tag="ev_t")
                nc.vector.tensor_scalar_add(t[:], ps[:], bm2_t[:, mt:mt + 1])
                nc.vector.tensor_tensor(f3[:, mt, :], t[:], f2[:, mt, :], OP.add)
            _dense(nc, pools, mybir, u4, Wm2_t, 4, 4, mlp2_cb)

            y6 = pools["a1"].tile([128, 4, CT], f32, tag="yln")
            _ln_stats_apply(nc, pools, mybir, f3, y6, ones_t, eps_t)

            th = pools["a1"].tile([128, 2, CT], f32, tag="th")

            def ap1_cb(mt, ps):
                nc.scalar.activation(th[:, mt, :], ps[:], AF.Tanh,
                                     bias=ba1_t[:, mt:mt + 1])
            _dense(nc, pools, mybir, y6, Wa1_t, 4, 2, ap1_cb)

            ps_a = pools["psumS"].tile([128, CT], f32, tag="ln_ps_s")
            for kt2 in range(2):
                nc.tensor.matmul(ps_a[:], Wa2_t[:, kt2, :].bitcast(f32r),
                                 th[:, kt2, :].bitcast(f32r),
                                 start=(kt2 == 0), stop=(kt2 == 1))
            E = pools["spool"].tile([128, CT], f32, tag="E")
            nc.scalar.activation(E[:], ps_a[:], AF.Exp, bias=ba2_t[:])

            for kt in range(4):
                wt = pools["spool"].tile([128, CT], f32, tag="pool_wt")
                nc.vector.tensor_tensor(wt[:], y6[:, kt, :], E[:], OP.mult)
                red = pools["spool"].tile([128, 1], f32, tag="pool_red")
                nc.vector.tensor_reduce(red[:], wt[:], mybir.AxisListType.X, OP.add)
                nc.vector.tensor_tensor(pp[:, kt:kt + 1], pp[:, kt:kt + 1],
                                        red[:], OP.add)
            redE = pools["spool"].tile([128, 1], f32, tag="pool_red")
            nc.vector.tensor_reduce(redE[:], E[:], mybir.AxisListType.X, OP.add)
            nc.vector.tensor_tensor(pp[:, 4:5], pp[:, 4:5], redE[:], OP.add)

        nc.sync.dma_start(part[:, :], pp[:])
    nc.compile()
    return nc


# --------------------------------------------------------------------------
# host orchestration
# --------------------------------------------------------------------------

def _shuffle_index(m):
    u = np.arange(NWIN)
    s = np.arange(WS)
    return (1024 * s[None, :] + 128 * m + u[:, None]).reshape(-1)


def kernel(**inputs):
    from concourse.bass_utils import run_bass_kernel_spmd

    nch = int(os.environ.get("KERNEL_NCH", NCH))
    trace = bool(int(os.environ.get("KERNEL_TRACE", "0")))

    host = _host_prep(inputs)
    core_ids = list(range(NCORES))

    shared1 = {k: host[k] for k in
               ("W1", "b1", "Wq1", "bq1", "Wk1", "bk1", "Wv1", "Wp1", "bp1", "ones")}
    in_maps1 = []
    for m in range(NCORES):
        im = dict(shared1)
        im["featsT"] = host["featsT"][m]
        im["hbias"] = host["hbias"][m]
        in_maps1.append(im)

    if _cache.get("nch") != nch:
        _cache["phase1"] = _build_phase1(nch)
        _cache["phase2"] = None
        _cache["nch"] = nch

    import time as _time
    _t = _time.time()
    r1 = run_bass_kernel_spmd(_cache["phase1"], in_maps1, core_ids, trace=trace)
    kernel.last_wall_p1 = _time.time() - _t
    f1_cat = np.concatenate([r1.results[m]["f1"] for m in range(NCORES)], axis=1)
    kernel.last_exec_ns_p1 = r1.exec_time_ns

    if _cache.get("phase2") is None:
        _cache["phase2"] = _build_phase2(nch, host["ba2"])

    shared2 = {k: host[k] for k in
               ("Wq2", "bq2", "Wk2", "bk2", "Wv2", "Wp2", "bp2",
                "Wm1", "bm1", "Wm2", "bm2", "Wa1", "ba1", "Wa2bc", "ones")}
    in_maps2 = []
    for m in range(NCORES):
        im = dict(shared2)
        im["g"] = np.ascontiguousarray(f1_cat[:, _shuffle_index(m)])
        in_maps2.append(im)

    _t = _time.time()
    r2 = run_bass_kernel_spmd(_cache["phase2"], in_maps2, core_ids, trace=trace)
    kernel.last_wall_p2 = _time.time() - _t
    kernel.last_exec_ns_p2 = r2.exec_time_ns

    num = np.zeros(D, np.float64)
    den = 0.0
    for m in range(NCORES):
        pq = r2.results[m]["partial"]
        num += pq[:, 0:4].T.reshape(D).astype(np.float64)
        den += float(pq[0, 4])
    pooled = host["g3"] * (num / den).astype(np.float32) + host["b3"]
    logits = pooled @ host["fc2_w"] + host["fc2_b"]
    hazards = 1.0 / (1.0 + np.exp(-logits))
    S = np.cumprod(1.0 - hazards)
    Y_hat = np.argmax(logits)
    return (hazards[None, :].astype(np.float32),
            S[None, :].astype(np.float32),
            np.array([Y_hat], dtype=np.int32))


# revision 12
# speedup vs baseline: 5.1898x; 5.1898x over previous
"""HVTSurv forward pass on 8 Trainium2 NeuronCores.

Sharding: data-parallel over windows. Phase 1 (per core: contiguous shard of
6272 tokens = 128 windows): fc1+relu, LN1-folded window attention with
relative-position bias, proj, residual, gelu -> f1 shard. Host reshuffles f1
into shuffle-window order (pure data movement). Phase 2 (per core: its 128
shuffle windows): norm1-folded shuffle attention, proj+residual, norm2-folded
MLP, residual, LN3, attention-pooling partial sums. Host combines the 8
partials and applies the tiny 512x4 head.

On-chip: activations feature-major [C, T]; matmuls in float32r (1 cyc/row);
LayerNorm stats via ones-matmul broadcast; attention computed transposed
(attnT[j,i]) so softmax denominators come from ones-matmuls and the
normalization lands on the feature-major attention output.
"""

import os
import numpy as np

NCORES = 8
N = 50176
CIN = 1024
D = 512
NH = 8
WS = 49
SHIFT = 7
SCALE = (D // NH) ** -0.5
T = N // NCORES          # 6272 tokens per core
NWIN = T // WS           # 128 windows per core
CW = 8                   # windows per chunk
CT = CW * WS             # 392 tokens per chunk
NCH = NWIN // CW         # 16 chunks
HP = NH // 2             # 4 head pairs
AW = HP * WS             # 196 free width of one window's attnT block

_cache = {}
USE_F32R = bool(int(os.environ.get("KERNEL_F32R", "1")))


def _piecewise_index(rp, alpha=1.9, beta=1.9 * 4, gamma=1.9 * 6, shift=SHIFT):
    rpf = rp.astype(np.float32)
    rp_abs = np.abs(rpf)
    mask = rp_abs <= 2.0 * alpha
    safe = np.maximum(rp_abs, 1e-6)
    y = np.sign(rpf) * np.minimum(
        np.round(np.log(safe / alpha) / np.log(gamma / alpha) * (beta - 2 * alpha)),
        float(shift),
    )
    idx = np.where(mask, np.sign(rpf), y)
    return idx.astype(np.int32)


def _host_prep(inputs):
    f32 = np.float32
    data = np.asarray(inputs["data"], dtype=f32)
    coords = data[0, :, :2]
    feats = data[0, :, 2:]

    featsT = [np.ascontiguousarray(feats[m * T:(m + 1) * T].T) for m in range(NCORES)]

    table = np.asarray(inputs["wa_bias_table"], dtype=f32)
    cw = coords.reshape(N // WS, WS, 2)
    rel = (cw[:, :, None, :] - cw[:, None, :, :]).astype(np.int32)
    idx = np.abs(_piecewise_index(rel[..., 0])) + np.abs(_piecewise_index(rel[..., 1]))
    biasv = table[idx]  # [1024, 49i, 49j, 8]
    hbias = []
    for m in range(NCORES):
        bv = biasv[m * NWIN:(m + 1) * NWIN]
        hb = np.full((NWIN, 128, AW), -100.0, dtype=f32)
        for hp in range(HP):
            hb[:, 0:WS, hp * WS:(hp + 1) * WS] = bv[:, :, :, 2 * hp].transpose(0, 2, 1)
            hb[:, 64:64 + WS, hp * WS:(hp + 1) * WS] = \
                bv[:, :, :, 2 * hp + 1].transpose(0, 2, 1)
        hbias.append(np.ascontiguousarray(hb))

    def fold(g, b, W, bb, scale=1.0):
        g = np.asarray(g, f32); b = np.asarray(b, f32)
        W = np.asarray(W, f32); bb = np.asarray(bb, f32)
        return (np.ascontiguousarray((g[:, None] * W) * scale),
                ((b @ W + bb) * scale).astype(f32))

    qkv1 = np.asarray(inputs["wa_qkv_w"], f32)
    qkv1b = np.asarray(inputs["wa_qkv_b"], f32)
    Wq1, bq1 = fold(inputs["ln1_g"], inputs["ln1_b"], qkv1[:, :D], qkv1b[:D], SCALE)
    Wk1, bk1 = fold(inputs["ln1_g"], inputs["ln1_b"], qkv1[:, D:2 * D], qkv1b[D:2 * D])
    Wv1, bv1 = fold(inputs["ln1_g"], inputs["ln1_b"], qkv1[:, 2 * D:], qkv1b[2 * D:])
    Wp1 = np.ascontiguousarray(np.asarray(inputs["wa_proj_w"], f32))
    bp1 = (np.asarray(inputs["wa_proj_b"], f32) + bv1 @ Wp1).astype(f32)

    qkv2 = np.asarray(inputs["sa_qkv_w"], f32)
    qkv2b = np.asarray(inputs["sa_qkv_b"], f32)
    Wq2, bq2 = fold(inputs["norm1_g"], inputs["norm1_b"], qkv2[:, :D], qkv2b[:D], SCALE)
    Wk2, bk2 = fold(inputs["norm1_g"], inputs["norm1_b"], qkv2[:, D:2 * D], qkv2b[D:2 * D])
    Wv2, bv2 = fold(inputs["norm1_g"], inputs["norm1_b"], qkv2[:, 2 * D:], qkv2b[2 * D:])
    Wp2 = np.ascontiguousarray(np.asarray(inputs["sa_proj_w"], f32))
    bp2 = (np.asarray(inputs["sa_proj_b"], f32) + bv2 @ Wp2).astype(f32)

    Wm1, bm1 = fold(inputs["norm2_g"], inputs["norm2_b"], inputs["mlp_w1"], inputs["mlp_b1"])
    Wm2 = np.ascontiguousarray(np.asarray(inputs["mlp_w2"], f32))
    bm2 = np.asarray(inputs["mlp_b2"], f32)

    g3 = np.asarray(inputs["norm3_g"], f32)
    b3 = np.asarray(inputs["norm3_b"], f32)
    Wa1, ba1 = fold(g3, b3, inputs["ap_w1"], inputs["ap_b1"])
    Wa2bc = np.ascontiguousarray(
        np.tile(np.asarray(inputs["ap_w2"], f32), (1, 128)))
    ba2 = float(np.asarray(inputs["ap_b2"], f32).reshape(-1)[0])

    def pbias(b):
        return np.ascontiguousarray(b.reshape(-1, 128).T.astype(f32))

    return dict(
        featsT=featsT, hbias=hbias,
        W1=np.ascontiguousarray(np.asarray(inputs["fc1_w"], f32)),
        b1=pbias(np.asarray(inputs["fc1_b"], f32)),
        Wq1=Wq1, bq1=pbias(bq1), Wk1=Wk1, bk1=pbias(bk1), Wv1=Wv1,
        Wp1=Wp1, bp1=pbias(bp1),
        Wq2=Wq2, bq2=pbias(bq2), Wk2=Wk2, bk2=pbias(bk2), Wv2=Wv2,
        Wp2=Wp2, bp2=pbias(bp2),
        Wm1=Wm1, bm1=pbias(bm1), Wm2=Wm2, bm2=pbias(bm2),
        Wa1=Wa1, ba1=pbias(ba1), Wa2bc=Wa2bc, ba2=ba2,
        g3=g3, b3=b3,
        fc2_w=np.asarray(inputs["fc2_w"], f32),
        fc2_b=np.asarray(inputs["fc2_b"], f32),
        ones=np.ones((128, 128), dtype=f32),
    )


# --------------------------------------------------------------------------
# kernel builders
# --------------------------------------------------------------------------

def _ln_stats_apply(nc, pools, mybir, x, y, ones_t, eps_t):
    """y = (x - mu) / sqrt(var + eps), feature-major x [128, 4, CT]."""
    f32 = mybir.dt.float32
    f32r = mybir.dt.float32r if USE_F32R else mybir.dt.float32
    AF = mybir.ActivationFunctionType
    OP = mybir.AluOpType
    spool, psumS = pools["spool"], pools["psumS"]

    ps_s = psumS.tile([128, CT], f32, tag="ln_ps_s")
    for kt in range(4):
        nc.tensor.matmul(ps_s[:], ones_t[:].bitcast(f32r), x[:, kt, :].bitcast(f32r),
                         start=(kt == 0), stop=(kt == 3))
    ps_q = psumS.tile([128, CT], f32, tag="ln_ps_q")
    for kt in range(4):
        sq = spool.tile([128, CT], f32r, tag="ln_sq")
        nc.scalar.activation(sq[:], x[:, kt, :], AF.Square)
        nc.tensor.matmul(ps_q[:], ones_t[:].bitcast(f32r), sq[:].bitcast(f32r),
                         start=(kt == 0), stop=(kt == 3))
    mu = spool.tile([128, CT], f32, tag="ln_mu")
    nc.scalar.activation(mu[:], ps_s[:], AF.Copy, scale=1.0 / D)
    m2 = spool.tile([128, CT], f32, tag="ln_m2")
    nc.scalar.activation(m2[:], ps_q[:], AF.Copy, scale=1.0 / D)
    var = spool.tile([128, CT], f32, tag="ln_var")
    nc.vector.tensor_tensor(var[:], mu[:], mu[:], OP.mult)
    nc.vector.tensor_tensor(var[:], m2[:], var[:], OP.subtract)
    r1 = spool.tile([128, CT], f32, tag="ln_r1")
    nc.scalar.activation(r1[:], var[:], AF.Ln, bias=eps_t[:])
    nc.scalar.activation(r1[:], r1[:], AF.Exp, scale=-0.5)
    r2 = spool.tile([128, CT], f32, tag="ln_r2")
    nc.vector.tensor_tensor(r2[:], mu[:], r1[:], OP.mult)
    for kt in range(4):
        nc.vector.tensor_tensor(y[:, kt, :], x[:, kt, :], r1[:], OP.mult)
        nc.vector.tensor_tensor(y[:, kt, :], y[:, kt, :], r2[:], OP.subtract)


def _dense(nc, pools, mybir, x, W_t, nk, nm, out_cb):
    """out[mt] = sum_kt W[:,kt,mt-tile].T @ x[:,kt,:] ; evict via out_cb."""
    f32 = mybir.dt.float32
    f32r = mybir.dt.float32r if USE_F32R else mybir.dt.float32
    psumM = pools["psumM"]
    for mt in range(nm):
        ps = psumM.tile([128, CT], f32, tag="dense_ps")
        for kt in range(nk):
            nc.tensor.matmul(
                ps[:], W_t[:, kt, mt * 128:(mt + 1) * 128].bitcast(f32r),
                x[:, kt, :].bitcast(f32r), start=(kt == 0), stop=(kt == nk - 1))
        out_cb(mt, ps)


def _attn_block(nc, pools, mybir, y, names, hb_tile, f_out_cb):
    """q/k/v + windowed attnT + PV + proj for one chunk.

    y: LN'd input [128,4,CT]. hb_tile: [128, CW, AW] rel-bias (or None).
    f_out_cb(mt, ps): consume proj psum."""
    f32 = mybir.dt.float32
    f32r = mybir.dt.float32r if USE_F32R else mybir.dt.float32
    AF = mybir.ActivationFunctionType
    OP = mybir.AluOpType
    a1, spool = pools["a1"], pools["spool"]
    psumA, psumD, psumV = pools["psumA"], pools["psumD"], pools["psumV"]
    ones_t = names["ones"]

    q4 = a1.tile([128, 4, CT], f32r, tag="q4")
    k4 = a1.tile([128, 4, CT], f32r, tag="k4")
    for dst, Wt, bt in ((q4, names["Wq"], names["bq"]), (k4, names["Wk"], names["bk"])):
        def cb(mt, ps, dst=dst, bt=bt):
            nc.scalar.activation(dst[:, mt, :], ps[:], AF.Identity,
                                 bias=bt[:, mt:mt + 1])
        _dense(nc, pools, mybir, y, Wt, 4, 4, cb)

    # v token-major, duplicated at partition rows 0-48 and 64-112
    vt_all = a1.tile([128, CW, 512], f32r, tag="vt_all")
    for w in range(CW):
        ps_v = psumV.tile([128, 512], f32, tag="ps_v")
        for kt in range(4):
            nc.tensor.matmul(ps_v[0:WS, :],
                             y[:, kt, w * WS:(w + 1) * WS].bitcast(f32r),
                             names["Wv"][:, kt, :].bitcast(f32r),
                             start=(kt == 0), stop=(kt == 3))
        nc.vector.tensor_copy(vt_all[0:WS, w, :], ps_v[0:WS, :])
        nc.sync.dma_start(vt_all[64:64 + WS, w, :], vt_all[0:WS, w, :])

    # qk -> attnT[j,i]; evict (+bias) to contiguous attn_s; batched exp
    attn_s = a1.tile([128, CW * AW], f32, tag="attn_s")
    for w in range(CW):
        a_ps = psumA.tile([128, AW], f32, tag="a_ps")
        ws = w * WS
        for hp in range(HP):
            nc.tensor.matmul(a_ps[0:WS, hp * WS:(hp + 1) * WS],
                             k4[0:64, hp, ws:ws + WS].bitcast(f32),
                             q4[0:64, hp, ws:ws + WS].bitcast(f32),
                             start=True, stop=True, tile_position=(0, 0))
            nc.tensor.matmul(a_ps[64:64 + WS, hp * WS:(hp + 1) * WS],
                             k4[64:128, hp, ws:ws + WS].bitcast(f32),
                             q4[64:128, hp, ws:ws + WS].bitcast(f32),
                             start=True, stop=True, tile_position=(64, 64))
        sl = attn_s[0:113, w * AW:(w + 1) * AW]
        if hb_tile is not None:
            nc.vector.tensor_tensor(sl, a_ps[0:113, :], hb_tile[0:113, w, :], OP.add)
        else:
            nc.vector.tensor_copy(sl, a_ps[0:113, :])

    p_s = a1.tile([128, CW * AW], f32r, tag="p_s")
    nc.scalar.activation(p_s[0:113, :], attn_s[0:113, :], AF.Exp)

    ao_n = names["ao_n"]
    for w in range(CW):
        wc = w * AW
        d_ps = psumD.tile([128, AW], f32, tag="d_ps")
        nc.tensor.matmul(d_ps[0:64, :], ones_t[0:WS, 0:64].bitcast(f32),
                         p_s[0:WS, wc:wc + AW].bitcast(f32),
                         start=True, stop=True, tile_position=(0, 0))
        nc.tensor.matmul(d_ps[64:128, :], ones_t[64:64 + WS, 0:64].bitcast(f32),
                         p_s[64:64 + WS, wc:wc + AW].bitcast(f32),
                         start=True, stop=True, tile_position=(64, 64))
        rden = spool.tile([128, AW], f32, tag="rden")
        nc.vector.reciprocal_approx_fast(rden[:], d_ps[:])

        ao_ps = psumA.tile([128, AW], f32, tag="ao_ps")
        for hp in range(HP):
            nc.tensor.matmul(ao_ps[0:64, hp * WS:(hp + 1) * WS],
                             vt_all[0:WS, w, hp * 128:hp * 128 + 64].bitcast(f32),
                             p_s[0:WS, wc + hp * WS:wc + (hp + 1) * WS].bitcast(f32),
                             start=True, stop=True, tile_position=(0, 0))
            nc.tensor.matmul(ao_ps[64:128, hp * WS:(hp + 1) * WS],
                             vt_all[64:64 + WS, w, hp * 128 + 64:hp * 128 + 128].bitcast(f32),
                             p_s[64:64 + WS, wc + hp * WS:wc + (hp + 1) * WS].bitcast(f32),
                             start=True, stop=True, tile_position=(64, 64))
        nc.vector.tensor_tensor(
            ao_n[:, :, w * WS:(w + 1) * WS],
            ao_ps[:].rearrange("p (hp i) -> p hp i", hp=HP),
            rden[:].rearrange("p (hp i) -> p hp i", hp=HP),
            OP.mult)

    _dense(nc, pools, mybir, ao_n, names["Wp"], 4, 4, f_out_cb)


def _mk_pools(tc, stack):
    p = {}
    p["wpool"] = stack.enter_context(tc.tile_pool(name="wpool", bufs=1))
    p["a2"] = stack.enter_context(tc.tile_pool(name="a2", bufs=2))
    p["a1"] = stack.enter_context(tc.tile_pool(name="a1", bufs=1))
    p["spool"] = stack.enter_context(tc.tile_pool(name="spool", bufs=2))
    p["psumM"] = stack.enter_context(tc.tile_pool(name="psumM", bufs=2, space="PSUM"))
    p["psumS"] = stack.enter_context(tc.tile_pool(name="psumS", bufs=1, space="PSUM"))
    p["psumA"] = stack.enter_context(tc.tile_pool(name="psumA", bufs=1, space="PSUM"))
    p["psumD"] = stack.enter_context(tc.tile_pool(name="psumD", bufs=1, space="PSUM"))
    p["psumV"] = stack.enter_context(tc.tile_pool(name="psumV", bufs=1, space="PSUM"))
    return p


def _loaders(nc, pools, mybir):
    f32 = mybir.dt.float32
    mmdt = mybir.dt.float32r if USE_F32R else mybir.dt.float32
    wpool = pools["wpool"]

    def wload(dram, ko, m):
        t = wpool.tile([128, ko, m], mmdt, tag=dram.name)
        nc.sync.dma_start(t[:], dram.ap().rearrange("(ko p) m -> p ko m", p=128))
        return t

    def bload(dram, cols, dt=None):
        t = wpool.tile([128, cols], dt or f32, tag=dram.name)
        nc.sync.dma_start(t[:], dram[:, :])
        return t

    return wload, bload


def _build_phase1(nch):
    from concourse import bacc
    import concourse.tile as tile
    import concourse.mybir as mybir
    from contextlib import ExitStack
    f32 = mybir.dt.float32
    AF = mybir.ActivationFunctionType
    OP = mybir.AluOpType

    nc = bacc.Bacc("TRN2", target_bir_lowering=False, debug=False,
                   num_devices=NCORES)
    mmdt = mybir.dt.float32r if USE_F32R else mybir.dt.float32
    featsT = nc.dram_tensor("featsT", [CIN, T], mmdt, kind="ExternalInput")
    hbias = nc.dram_tensor("hbias", [NWIN, 128, AW], f32, kind="ExternalInput")
    dr = {}
    for nm, shp in (("W1", [CIN, D]), ("b1", [128, 4]),
                    ("Wq1", [D, D]), ("bq1", [128, 4]),
                    ("Wk1", [D, D]), ("bk1", [128, 4]),
                    ("Wv1", [D, D]),
                    ("Wp1", [D, D]), ("bp1", [128, 4]),
                    ("ones", [128, 128])):
        dt_ = f32 if nm.startswith("b") else mmdt
        dr[nm] = nc.dram_tensor(nm, shp, dt_, kind="ExternalInput")
    f1out = nc.dram_tensor("f1", [D, T], f32, kind="ExternalOutput")

    with tile.TileContext(nc) as tc, ExitStack() as stack:
        pools = _mk_pools(tc, stack)
        wload, bload = _loaders(nc, pools, mybir)
        W1_t = wload(dr["W1"], 8, D)
        Wq_t = wload(dr["Wq1"], 4, D)
        Wk_t = wload(dr["Wk1"], 4, D)
        Wv_t = wload(dr["Wv1"], 4, D)
        Wp_t = wload(dr["Wp1"], 4, D)
        b1_t = bload(dr["b1"], 4)
        bq_t = bload(dr["bq1"], 4)
        bk_t = bload(dr["bk1"], 4)
        bp_t = bload(dr["bp1"], 4)
        ones_t = bload(dr["ones"], 128, mmdt)
        eps_t = pools["wpool"].tile([128, 1], f32, tag="eps")
        nc.vector.memset(eps_t[:], 1e-5)

        fT3 = featsT.ap().rearrange("(ko p) t -> p ko t", p=128)
        f1o = f1out.ap().rearrange("(ko p) t -> p ko t", p=128)

        for c in range(nch):
            cs = c * CT
            x8 = pools["a2"].tile([128, 8, CT], mmdt, tag="x8")
            nc.sync.dma_start(x8[:], fT3[:, :, cs:cs + CT])
            hb_tile = pools["a1"].tile([128, CW, AW], f32, tag="hb")
            nc.sync.dma_start(
                hb_tile[:],
                hbias.ap()[c * CW:(c + 1) * CW, :, :].rearrange("w p a -> p w a"))

            h4 = pools["a2"].tile([128, 4, CT], mmdt, tag="h4")

            def relu_cb(mt, ps):
                nc.scalar.activation(h4[:, mt, :], ps[:], AF.Relu,
                                     bias=b1_t[:, mt:mt + 1])
            _dense(nc, pools, mybir, x8, W1_t, 8, 4, relu_cb)

            y4 = pools["a1"].tile([128, 4, CT], mmdt, tag="y4")
            _ln_stats_apply(nc, pools, mybir, h4, y4, ones_t, eps_t)

            ao_n = pools["a1"].tile([128, 4, CT], mmdt, tag="ao_n")
            f1c = pools["a1"].tile([128, 4, CT], f32, tag="f1c")

            def f_cb(mt, ps):
                hp_t = pools["spool"].tile([128, CT], f32, tag="hp_t")
                nc.vector.tensor_tensor(hp_t[:], ps[:], h4[:, mt, :], OP.add)
                nc.scalar.activation(f1c[:, mt, :], hp_t[:], AF.Gelu,
                                     bias=bp_t[:, mt:mt + 1])

            names = dict(Wq=Wq_t, bq=bq_t, Wk=Wk_t, bk=bk_t, Wv=Wv_t,
                         Wp=Wp_t, ones=ones_t, ao_n=ao_n)
            _attn_block(nc, pools, mybir, y4, names, hb_tile, f_cb)

            nc.sync.dma_start(f1o[:, :, cs:cs + CT], f1c[:])
    nc.compile()
    return nc


def _build_phase2(nch, ba2):
    from concourse import bacc
    import concourse.tile as tile
    import concourse.mybir as mybir
    from contextlib import ExitStack
    f32 = mybir.dt.float32
    f32r = mybir.dt.float32r if USE_F32R else mybir.dt.float32
    AF = mybir.ActivationFunctionType
    OP = mybir.AluOpType

    nc = bacc.Bacc("TRN2", target_bir_lowering=False, debug=False,
                   num_devices=NCORES)
    mmdt = mybir.dt.float32r if USE_F32R else mybir.dt.float32
    g_in = nc.dram_tensor("g", [D, T], mmdt, kind="ExternalInput")
    dr = {}
    for nm, shp in (("Wq2", [D, D]), ("bq2", [128, 4]),
                    ("Wk2", [D, D]), ("bk2", [128, 4]),
                    ("Wv2", [D, D]),
                    ("Wp2", [D, D]), ("bp2", [128, 4]),
                    ("Wm1", [D, D]), ("bm1", [128, 4]),
                    ("Wm2", [D, D]), ("bm2", [128, 4]),
                    ("Wa1", [D, 256]), ("ba1", [128, 2]),
                    ("Wa2bc", [256, 128]),
                    ("ones", [128, 128])):
        dt_ = f32 if nm.startswith("b") else mmdt
        dr[nm] = nc.dram_tensor(nm, shp, dt_, kind="ExternalInput")
    part = nc.dram_tensor("partial", [128, 8], f32, kind="ExternalOutput")

    with tile.TileContext(nc) as tc, ExitStack() as stack:
        pools = _mk_pools(tc, stack)
        wload, bload = _loaders(nc, pools, mybir)
        Wq_t = wload(dr["Wq2"], 4, D)
        Wk_t = wload(dr["Wk2"], 4, D)
        Wv_t = wload(dr["Wv2"], 4, D)
        Wp_t = wload(dr["Wp2"], 4, D)
        Wm1_t = wload(dr["Wm1"], 4, D)
        Wm2_t = wload(dr["Wm2"], 4, D)
        Wa1_t = wload(dr["Wa1"], 4, 256)
        Wa2_t = wload(dr["Wa2bc"], 2, 128)
        bq_t = bload(dr["bq2"], 4)
        bk_t = bload(dr["bk2"], 4)
        bp_t = bload(dr["bp2"], 4)
        bm1_t = bload(dr["bm1"], 4)
        bm2_t = bload(dr["bm2"], 4)
        ba1_t = bload(dr["ba1"], 2)
        ones_t = bload(dr["ones"], 128, mmdt)
        eps_t = pools["wpool"].tile([128, 1], f32, tag="eps")
        nc.vector.memset(eps_t[:], 1e-5)
        ba2_t = pools["wpool"].tile([128, 1], f32, tag="ba2")
        nc.vector.memset(ba2_t[:], float(ba2))
        pp = pools["wpool"].tile([128, 8], f32, tag="pp")
        nc.vector.memset(pp[:], 0.0)

        gT3 = g_in.ap().rearrange("(ko p) t -> p ko t", p=128)

        for c in range(nch):
            cs = c * CT
            g4 = pools["a2"].tile([128, 4, CT], mmdt, tag="g4")
            nc.sync.dma_start(g4[:], gT3[:, :, cs:cs + CT])

            y4 = pools["a1"].tile([128, 4, CT], mmdt, tag="yln")
            _ln_stats_apply(nc, pools, mybir, g4, y4, ones_t, eps_t)

            ao_n = pools["a1"].tile([128, 4, CT], mmdt, tag="ao_n")
            f2 = pools["a2"].tile([128, 4, CT], mmdt, tag="f2")

            def f2_cb(mt, ps):
                t = pools["spool"].tile([128, CT], f32, tag="ev_t")
                nc.vector.tensor_scalar_add(t[:], ps[:], bp_t[:, mt:mt + 1])
                nc.vector.tensor_tensor(f2[:, mt, :], t[:], g4[:, mt, :], OP.add)

            names = dict(Wq=Wq_t, bq=bq_t, Wk=Wk_t, bk=bk_t, Wv=Wv_t,
                         Wp=Wp_t, ones=ones_t, ao_n=ao_n)
            _attn_block(nc, pools, mybir, y4, names, None, f2_cb)

            y5 = pools["a1"].tile([128, 4, CT], mmdt, tag="yln")
            _ln_stats_apply(nc, pools, mybir, f2, y5, ones_t, eps_t)
            u4 = pools["a1"].tile([128, 4, CT], mmdt, tag="u4")

            def mlp1_cb(mt, ps):
                nc.scalar.activation(u4[:, mt, :], ps[:], AF.Gelu,
                                     bias=bm1_t[:, mt:mt + 1])
            _dense(nc, pools, mybir, y5, Wm1_t, 4, 4, mlp1_cb)

            f3 = pools["a2"].tile([128, 4, CT], mmdt, tag="f3")

            def mlp2_cb(mt, ps):
                t = pools["spool"].tile([128, CT], f32, tag="ev_t")
                nc.vector.tensor_scalar_add(t[:], ps[:], bm2_t[:, mt:mt + 1])
                nc.vector.tensor_tensor(f3[:, mt, :], t[:], f2[:, mt, :], OP.add)
            _dense(nc, pools, mybir, u4, Wm2_t, 4, 4, mlp2_cb)

            y6 = pools["a1"].tile([128, 4, CT], mmdt, tag="yln")
            _ln_stats_apply(nc, pools, mybir, f3, y6, ones_t, eps_t)

            th = pools["a1"].tile([128, 2, CT], mmdt, tag="th")

            def ap1_cb(mt, ps):
                nc.scalar.activation(th[:, mt, :], ps[:], AF.Tanh,
                                     bias=ba1_t[:, mt:mt + 1])
            _dense(nc, pools, mybir, y6, Wa1_t, 4, 2, ap1_cb)

            ps_a = pools["psumS"].tile([128, CT], f32, tag="ln_ps_s")
            for kt2 in range(2):
                nc.tensor.matmul(ps_a[:], Wa2_t[:, kt2, :].bitcast(f32r),
                                 th[:, kt2, :].bitcast(f32r),
                                 start=(kt2 == 0), stop=(kt2 == 1))
            E = pools["spool"].tile([128, CT], f32, tag="E")
            nc.scalar.activation(E[:], ps_a[:], AF.Exp, bias=ba2_t[:])

            for kt in range(4):
                wt = pools["spool"].tile([128, CT], f32, tag="pool_wt")
                nc.vector.tensor_tensor(wt[:], y6[:, kt, :], E[:], OP.mult)
                red = pools["spool"].tile([128, 1], f32, tag="pool_red")
                nc.vector.tensor_reduce(red[:], wt[:], mybir.AxisListType.X, OP.add)
                nc.vector.tensor_tensor(pp[:, kt:kt + 1], pp[:, kt:kt + 1],
                                        red[:], OP.add)
            redE = pools["spool"].tile([128, 1], f32, tag="pool_red")
            nc.vector.tensor_reduce(redE[:], E[:], mybir.AxisListType.X, OP.add)
            nc.vector.tensor_tensor(pp[:, 4:5], pp[:, 4:5], redE[:], OP.add)

        nc.sync.dma_start(part[:, :], pp[:])
    nc.compile()
    return nc


# --------------------------------------------------------------------------
# host orchestration
# --------------------------------------------------------------------------

def _shuffle_index(m):
    u = np.arange(NWIN)
    s = np.arange(WS)
    return (1024 * s[None, :] + 128 * m + u[:, None]).reshape(-1)


def kernel(**inputs):
    from concourse.bass_utils import run_bass_kernel_spmd

    nch = int(os.environ.get("KERNEL_NCH", NCH))
    trace = bool(int(os.environ.get("KERNEL_TRACE", "0")))

    host = _host_prep(inputs)
    core_ids = list(range(NCORES))

    shared1 = {k: host[k] for k in
               ("W1", "b1", "Wq1", "bq1", "Wk1", "bk1", "Wv1", "Wp1", "bp1", "ones")}
    in_maps1 = []
    for m in range(NCORES):
        im = dict(shared1)
        im["featsT"] = host["featsT"][m]
        im["hbias"] = host["hbias"][m]
        in_maps1.append(im)

    if _cache.get("nch") != nch:
        _cache["phase1"] = _build_phase1(nch)
        _cache["phase2"] = None
        _cache["nch"] = nch

    import time as _time
    _t = _time.time()
    r1 = run_bass_kernel_spmd(_cache["phase1"], in_maps1, core_ids, trace=trace)
    kernel.last_wall_p1 = _time.time() - _t
    f1_cat = np.concatenate([r1.results[m]["f1"] for m in range(NCORES)], axis=1)
    kernel.last_exec_ns_p1 = r1.exec_time_ns

    if _cache.get("phase2") is None:
        _cache["phase2"] = _build_phase2(nch, host["ba2"])

    shared2 = {k: host[k] for k in
               ("Wq2", "bq2", "Wk2", "bk2", "Wv2", "Wp2", "bp2",
                "Wm1", "bm1", "Wm2", "bm2", "Wa1", "ba1", "Wa2bc", "ones")}
    in_maps2 = []
    for m in range(NCORES):
        im = dict(shared2)
        im["g"] = np.ascontiguousarray(f1_cat[:, _shuffle_index(m)])
        in_maps2.append(im)

    _t = _time.time()
    r2 = run_bass_kernel_spmd(_cache["phase2"], in_maps2, core_ids, trace=trace)
    kernel.last_wall_p2 = _time.time() - _t
    kernel.last_exec_ns_p2 = r2.exec_time_ns

    num = np.zeros(D, np.float64)
    den = 0.0
    for m in range(NCORES):
        pq = r2.results[m]["partial"]
        num += pq[:, 0:4].T.reshape(D).astype(np.float64)
        den += float(pq[0, 4])
    pooled = host["g3"] * (num / den).astype(np.float32) + host["b3"]
    logits = pooled @ host["fc2_w"] + host["fc2_b"]
    hazards = 1.0 / (1.0 + np.exp(-logits))
    S = np.cumprod(1.0 - hazards)
    Y_hat = np.argmax(logits)
    return (hazards[None, :].astype(np.float32),
            S[None, :].astype(np.float32),
            np.array([Y_hat], dtype=np.int32))
